# revision 35
# baseline (speedup 1.0000x reference)
"""GIN message-passing (3 layers + JumpingKnowledge cat + Linear) on 8 TRN2 NeuronCores.

Strategy (graph/data parallel, edges sharded by destination node):
  - Nodes are partitioned into 8 contiguous shards (12500 real + 44 pad rows per
    core, padded so each shard is 98 groups of 128).  Core c computes the
    aggregation + MLP for its own dst shard and gathers source features from a
    full local copy of h (replicated via AllGather between layers).
  - The segment_sum gather uses the GPSIMD dma_gather primitive (int16 indices,
    256B rows).  Since int16 only addresses 32768 rows, sources are split into
    4 windows of 32768 canonical positions; per (core, window) the dst nodes are
    sorted by in-degree and packed into groups of 128 with per-chunk-uniform
    slot counts, padding slots pointing at an all-zero row of h.
  - Gathered tiles [128 nodes, D slots, 64ch] are segment-summed on the Vector
    engine (strided reduce over the slot axis), assembled per window, and
    scatter-added (unique indices per call -> no RMW collisions) back into
    canonical shard order in DRAM.
  - MLP runs channel-major on the TensorEngine (transpose via PE identity
    matmul), biases+ReLU on the Scalar engine.  The JumpingKnowledge concat +
    final Linear is computed as a 3-matmul PSUM accumulation over the per-layer
    shard outputs.
"""

import os
import sys

os.environ.setdefault("MYCRO_LOCAL_CACHE", "1")
if "/opt/trn_rl_repo" not in sys.path:
    sys.path.insert(0, "/opt/trn_rl_repo")

from contextlib import ExitStack
from dataclasses import dataclass, field

import numpy as np


# --------------------------------------------------------------------------- #
# configuration
# --------------------------------------------------------------------------- #
@dataclass
class Cfg:
    n: int = 100000          # real nodes
    e: int = 1600000
    c: int = 64              # channels (in == hid == 64)
    ncores: int = 8
    window: int = 32768      # int16-addressable row window
    cols_max: int = 96       # gather-chunk column budget (slots per partition)
    qcols: int = 8          # idx columns per SWDGE gather call (128*qcols idxs)
    scratch: int = 16384     # SWDGE descriptor carveout bytes/partition
    tile_n: int = 512        # MLP node-tile width
    n_layers: int = 3
    nqueues: int = 4         # SWDGE queues; round-robin unlocks SDMA parallelism
    ag_bf16: bool = True     # AllGather in bf16 with cast DMAs around it
    nsplit: int = 4          # >1: shard quarters define windows; AG per quarter
    jk_fuse: bool = True     # fold the final Linear into layer-2's MLP tiles

    nsh_real: int = field(init=False)
    groups: int = field(init=False)
    nsh: int = field(init=False)
    ntot: int = field(init=False)
    nwin: int = field(init=False)
    qreal: int = field(init=False)
    qrows: int = field(init=False)
    wlen: int = field(init=False)

    def __post_init__(self):
        assert self.n % self.ncores == 0
        self.nsh_real = self.n // self.ncores
        self.groups = -(-self.nsh_real // 128)
        self.nsh = self.groups * 128
        self.ntot = self.nsh * self.ncores
        if self.nsplit > 1:
            S = self.nsplit
            assert self.nsh_real % S == 0 and self.nsh % S == 0
            self.qreal = self.nsh_real // S
            self.qrows = self.nsh // S
            self.wlen = self.ncores * self.qrows
            assert self.wlen <= 32768, "window must stay int16-addressable"
            assert self.qrows > self.qreal, "each quarter needs a zero row"
            self.nwin = S
        else:
            self.qreal = self.qrows = 0
            self.wlen = self.window
            self.nwin = -(-self.ntot // self.window)
        assert self.nsh % 16 == 0
        assert self.nsh > self.nsh_real, "need pad rows to host the zero rows"
        assert self.tile_n % 128 == 0


@dataclass
class Sched:
    """Cross-core-uniform gather schedule.

    chunks[w] = list of (g0, [D_g ...]) — consecutive active groups packed into
    one SBUF tile; gather calls are split along the chunk's columns in pieces
    of <= QCOLS (descriptor-ring cap).  gmax[w] = number of active groups.
    """
    chunks: list
    gmax: list
    totc: list    # per window: total idx columns (sum nidx/16)
    zr: list      # per window: absolute canonical position of an all-zero row


# --------------------------------------------------------------------------- #
# host-side preprocessing
# --------------------------------------------------------------------------- #
def _wrap_idx(v: np.ndarray) -> np.ndarray:
    """int16 vector (len % 16 == 0) -> [128, len/16] wrapped+replicated layout.

    Logical index j lives at [j % 16, j // 16]; the 16-partition pattern is
    replicated to all 128 partitions (ucode cores each read their own stripe).
    """
    w16 = v.reshape(-1, 16).T  # [16, len/16]
    return np.ascontiguousarray(np.tile(w16, (8, 1)))


def _local_row(j: np.ndarray, cfg: Cfg):
    """shard-local node index -> shard row (quarter layout when nsplit>1)."""
    if cfg.nsplit > 1:
        return (j // cfg.qreal) * cfg.qrows + j % cfg.qreal
    return j


def preprocess(edge_index: np.ndarray, cfg: Cfg):
    nc_, nw, nsh = cfg.ncores, cfg.nwin, cfg.nsh
    wlen = cfg.wlen
    src = edge_index[0].astype(np.int64)
    dst = edge_index[1].astype(np.int64)

    c_of = dst // cfg.nsh_real
    ld = _local_row(dst % cfg.nsh_real, cfg)                  # local dst row
    if cfg.nsplit > 1:
        # canonical pos: [quarter][core][row-in-quarter]
        cs = src // cfg.nsh_real
        j = src % cfg.nsh_real
        ps = (j // cfg.qreal) * wlen + cs * cfg.qrows + j % cfg.qreal
    else:
        ps = (src // cfg.nsh_real) * nsh + src % cfg.nsh_real
    w_of = ps // wlen

    key = (c_of * nw + w_of) * nsh + ld
    order = np.lexsort((ps, key))
    key_s = key[order]
    ps_s = ps[order]
    cnt = np.bincount(key_s, minlength=nc_ * nw * nsh).reshape(nc_, nw, nsh)
    base = np.zeros(nc_ * nw * nsh + 1, np.int64)
    base[1:] = np.cumsum(cnt.ravel())

    # window row counts + zero rows (pad rows stay zero every layer)
    if cfg.nsplit > 1:
        wrows = [wlen] * nw
        pad_pos = np.concatenate(
            [
                q * wlen + c * cfg.qrows + np.arange(cfg.qreal, cfg.qrows)
                for q in range(cfg.nsplit)
                for c in range(nc_)
            ]
        )
    else:
        wrows = [min(wlen, cfg.ntot - w * wlen) for w in range(nw)]
        pad_pos = np.concatenate(
            [np.arange(c * nsh + cfg.nsh_real, (c + 1) * nsh) for c in range(nc_)]
        )
    zr = []
    for w in range(nw):
        cand = pad_pos[(pad_pos >= w * wlen) & (pad_pos < w * wlen + wrows[w])]
        assert cand.size, f"window {w} has no zero row"
        zr.append(int(cand[0]))

    # per-(core,window) degree-sorted node order
    pi = np.empty((nc_, nw, nsh), np.int64)
    for c in range(nc_):
        for w in range(nw):
            pi[c, w] = np.argsort(-cnt[c, w], kind="stable")

    # shared schedule: per-group D = cross-core max of group max degree
    D = np.zeros((nw, cfg.groups), np.int64)
    for w in range(nw):
        sd = -np.sort(-cnt[:, w, :], axis=1)       # per-core sorted desc
        D[w] = sd[:, ::128].max(axis=0)
    chunks, gmax, totc = [], [], []
    for w in range(nw):
        gm = int(np.count_nonzero(D[w]))           # actives are a prefix
        ch, g0 = [], 0
        while g0 < gm:
            ds, g1 = [], g0
            while g1 < gm and sum(ds) + D[w][g1] <= cfg.cols_max:
                ds.append(int(D[w][g1]))
                g1 += 1
            if not ds:  # single group larger than cols_max
                ds = [int(D[w][g0])]
                g1 = g0 + 1
            # pad to a multiple of 8 columns so every SWDGE call is a full
            # 1024 descriptors (padding columns gather the zero row)
            padc = (-sum(ds)) % 8
            ch.append((g0, ds, padc))
            g0 = g1
        chunks.append(ch)
        gmax.append(gm)
        totc.append(sum(128 * (sum(ds) + padc) // 16 for (_, ds, padc) in ch))

    # per-core gather / scatter index arrays
    gidx = [[None] * nw for _ in range(nc_)]
    sidx = [[None] * nw for _ in range(nc_)]
    tot_slots = 0
    for c in range(nc_):
        for w in range(nw):
            parts = []
            zr_loc = zr[w] - w * wlen
            for (g0, ds, padc) in chunks[w]:
                cols = sum(ds)
                nidx = 128 * cols
                col2g = np.repeat(np.arange(len(ds)), ds)         # chunk-local g
                col2d = np.concatenate([np.arange(dd) for dd in ds])
                i = np.arange(nidx)
                p = i & 127
                col = i >> 7
                r = (g0 + col2g[col]) * 128 + p
                d = col2d[col]
                node = pi[c, w][r]
                deg = cnt[c, w, node]
                k = (c * nw + w) * nsh + node
                e = base[k] + np.minimum(d, np.maximum(deg - 1, 0))
                e = np.minimum(e, max(len(ps_s) - 1, 0))  # empty tail segments
                v = ps_s[e] - w * wlen
                v = np.where(d < deg, v, zr_loc)
                if padc:
                    v = np.concatenate([v, np.full(128 * padc, zr_loc, np.int64)])
                assert v.min() >= 0 and v.max() < wrows[w]
                parts.append(_wrap_idx(v.astype(np.int16)))
                tot_slots += nidx + 128 * padc
            gidx[c][w] = np.concatenate(parts, axis=1)
            rank = np.empty(nsh, np.int64)
            rank[pi[c, w]] = np.arange(nsh)  # node -> token position
            sidx[c][w] = _wrap_idx(rank.astype(np.int16))

    pad_frac = tot_slots / max(len(src), 1)
    return Sched(chunks, gmax, totc, zr), gidx, sidx, wrows, pad_frac


# --------------------------------------------------------------------------- #
# device program
# --------------------------------------------------------------------------- #
def _patch_queue_affine_sem_lanes():
    """Make Tile's DMASW lane assignment queue-affine.

    SWDGE completion sems are locked to the queue that first increments them
    (ucode sem_target is per-queue).  Tile cycles lanes 0..7 in scheduled
    order, which breaks once calls use queue_num 1..3.  Map queue q to lanes
    {q, q+4} so every lane only ever serves one queue.
    """
    import concourse.tile_sem_assignment as tsa
    import concourse.bass_isa as bass_isa
    import concourse.mybir as mybir

    if getattr(tsa, "_queue_affine_patched", False):
        return
    orig = tsa.TileClockTick._assign_tick
    DMAInst = tsa.DMAInst

    def _assign_tick(self, inst):
        if (
            inst.engine == mybir.EngineType.Pool
            and isinstance(inst, DMAInst)
            and not isinstance(inst, bass_isa.UserSyncedRemoteDMADescs)
            and self.swdge_sem_count == 8
        ):
            q = int(getattr(inst, "queue_num", 0) or 0) % 4
            tog = getattr(self, "_q_lane_toggle", None)
            if tog is None:
                tog = self._q_lane_toggle = {}
            t = tog.get(q, 0)
            tog[q] = t ^ 1
            self.next_sw_dma_idx = q + 4 * t
        return orig(self, inst)

    tsa.TileClockTick._assign_tick = _assign_tick
    tsa._queue_affine_patched = True


def build_program(cfg: Cfg, sched: Sched, wrows, debug=None, reps=1):
    import concourse.bacc as bacc
    import concourse.mybir as mybir
    import concourse.tile as tile
    from concourse.masks import make_identity

    if cfg.nqueues > 1:
        _patch_queue_affine_sem_lanes()

    f32 = mybir.dt.float32
    i16 = mybir.dt.int16
    C = cfg.c
    P = 128
    NL = cfg.n_layers

    nc = bacc.Bacc(
        "TRN2",
        target_bir_lowering=False,
        debug=False,
        num_devices=cfg.ncores,
        num_swdge_queues=cfg.nqueues,
        dynamic_dma_scratch_size=cfg.scratch,
    )
    qrr = iter(range(10**9))  # round-robin SWDGE queue counter

    def nextq():
        return next(qrr) % cfg.nqueues

    x_full = nc.dram_tensor("x_full", [cfg.ntot, C], f32, kind="ExternalInput")
    x_shard = nc.dram_tensor("x_shard", [cfg.nsh, C], f32, kind="ExternalInput")
    gidx_t = [
        nc.dram_tensor(f"gidx_w{w}", [P, sched.totc[w]], i16, kind="ExternalInput")
        for w in range(cfg.nwin)
    ]
    sidx_t = [
        nc.dram_tensor(f"sidx_w{w}", [P, cfg.nsh // 16], i16, kind="ExternalInput")
        for w in range(cfg.nwin)
    ]
    w1_t, b1_t, w2_t, b2_t = [], [], [], []
    for l in range(NL):
        w1_t.append(nc.dram_tensor(f"W1_{l}", [C, C], f32, kind="ExternalInput"))
        b1_t.append(nc.dram_tensor(f"b1_{l}", [C, 1], f32, kind="ExternalInput"))
        w2_t.append(nc.dram_tensor(f"W2_{l}", [C, C], f32, kind="ExternalInput"))
        b2_t.append(nc.dram_tensor(f"b2_{l}", [C, 1], f32, kind="ExternalInput"))
    linw_t = nc.dram_tensor("lin_W", [NL * C, C], f32, kind="ExternalInput")
    linb_t = nc.dram_tensor("lin_b", [C, 1], f32, kind="ExternalInput")
    out_t = nc.dram_tensor("out_shard", [cfg.nsh, C], f32, kind="ExternalOutput")

    rg = [list(range(cfg.ncores))]
    add = mybir.AluOpType.add
    relu = mybir.ActivationFunctionType.Relu

    with tile.TileContext(nc) as tc, ExitStack() as ctx:
        const = ctx.enter_context(tc.tile_pool(name="const", bufs=1))
        idxp = ctx.enter_context(tc.tile_pool(name="idx", bufs=3))
        gat = ctx.enter_context(tc.tile_pool(name="gat", bufs=3))
        asmp = ctx.enter_context(tc.tile_pool(name="asm", bufs=2))
        mlp = ctx.enter_context(tc.tile_pool(name="mlp", bufs=2))
        psum = ctx.enter_context(tc.tile_pool(name="psum", bufs=2, space="PSUM"))
        dram = ctx.enter_context(tc.tile_pool(name="dram", bufs=1, space="DRAM"))

        # ---- constants ----
        identity = const.tile([P, P], f32)
        make_identity(nc, identity[:])
        ZCH = 16  # zero-fill chunk (groups per DMA)
        zeros = const.tile([P, ZCH * C], f32)
        nc.vector.memset(zeros[:], 0.0)
        w1s, b1s, w2s, b2s, lws = [], [], [], [], []
        for l in range(NL):
            w1 = const.tile([C, C], f32, name=f"w1s_{l}")
            nc.sync.dma_start(out=w1[:], in_=w1_t[l].ap())
            w1s.append(w1)
            b1 = const.tile([C, 1], f32, name=f"b1s_{l}")
            nc.sync.dma_start(out=b1[:], in_=b1_t[l].ap())
            b1s.append(b1)
            w2 = const.tile([C, C], f32, name=f"w2s_{l}")
            nc.sync.dma_start(out=w2[:], in_=w2_t[l].ap())
            w2s.append(w2)
            b2 = const.tile([C, 1], f32, name=f"b2s_{l}")
            nc.sync.dma_start(out=b2[:], in_=b2_t[l].ap())
            b2s.append(b2)
            lw = const.tile([C, C], f32, name=f"lws_{l}")
            nc.sync.dma_start(out=lw[:], in_=linw_t.ap()[l * C : (l + 1) * C, :])
            lws.append(lw)
        lb = const.tile([C, 1], f32)
        nc.sync.dma_start(out=lb[:], in_=linb_t.ap())
        sis = []  # node -> token rank per window (agg gather-back indices)
        for w in range(cfg.nwin):
            si = const.tile([P, cfg.nsh // 16], i16, name=f"sis_{w}")
            nc.sync.dma_start(out=si[:], in_=sidx_t[w].ap())
            sis.append(si)

        # ---- internal DRAM ----
        # per-window partial aggregations, in token (degree-sorted) order
        bufw = [dram.tile([cfg.nsh, C], f32, name=f"bufw_{w}") for w in range(cfg.nwin)]
        shard = [dram.tile([cfg.nsh, C], f32, name=f"shard_{l}") for l in range(NL)]
        # channel-major copies of layers 0/1 for the fused JK matmuls
        sh_cm = [
            dram.tile([C, cfg.nsh], f32, name=f"shcm_{l}") for l in range(NL)
        ] if cfg.jk_fuse else None
        # Shared tensors allow a single writer only -> one set per rep
        bf16 = mybir.dt.bfloat16
        hf_all = hf16_all = hfq_all = sh16 = None
        if cfg.nsplit > 1:
            agdt = bf16 if cfg.ag_bf16 else f32
            hfq_all = [
                [
                    [
                        dram.tile(
                            [cfg.wlen, C], agdt, name=f"hfq_{r}_{l}_{q}",
                            addr_space="Shared",
                        )
                        for q in range(cfg.nsplit)
                    ]
                    for l in range(NL - 1)
                ]
                for r in range(reps)
            ]
            if cfg.ag_bf16:
                hf_all = [
                    [
                        dram.tile([cfg.ntot, C], f32, name=f"hf_{r}_{l}")
                        for l in range(NL - 1)
                    ]
                    for r in range(reps)
                ]
                sh16 = [
                    dram.tile([cfg.nsh, C], bf16, name=f"sh16_{l}")
                    for l in range(NL - 1)
                ]
        elif cfg.ag_bf16:
            hf_all = [
                [
                    dram.tile([cfg.ntot, C], f32, name=f"hf_{r}_{l}")
                    for l in range(NL - 1)
                ]
                for r in range(reps)
            ]
            hf16_all = [
                [
                    dram.tile(
                        [cfg.ntot, C], bf16, name=f"hf16_{r}_{l}", addr_space="Shared"
                    )
                    for l in range(NL - 1)
                ]
                for r in range(reps)
            ]
            sh16 = [
                dram.tile([cfg.nsh, C], bf16, name=f"sh16_{l}") for l in range(NL - 1)
            ]
        else:
            hf_all = [
                [
                    dram.tile([cfg.ntot, C], f32, name=f"hf_{r}_{l}", addr_space="Shared")
                    for l in range(NL - 1)
                ]
                for r in range(reps)
            ]

        npad = cfg.nsh - cfg.nsh_real
        cc_full = cfg.tile_n // 128

        def transpose_in(src_ap, dst_ap, cc):
            """node-major [128, cc*C] -> channel-major [C, cc*128]."""
            for s in range(cc):
                pt = psum.tile([C, P], f32, name="tp", tag="tp")
                nc.tensor.transpose(
                    out=pt[:], in_=src_ap[:, s * C : (s + 1) * C], identity=identity[:]
                )
                nc.scalar.copy(out=dst_ap[:, s * P : (s + 1) * P], in_=pt[:])

        def transpose_out(src_ap, dst_ap, cc):
            """channel-major [C, cc*128] -> node-major [128, cc*C]."""
            for s in range(cc):
                pt = psum.tile([P, C], f32, name="tpo", tag="tp")
                nc.tensor.transpose(
                    out=pt[:],
                    in_=src_ap[:, s * P : (s + 1) * P],
                    identity=identity[:C, :C],
                )
                nc.scalar.copy(out=dst_ap[:, s * C : (s + 1) * C], in_=pt[:])

        def node_tiles():
            t0 = 0
            while t0 < cfg.nsh:
                tn = min(cfg.tile_n, cfg.nsh - t0)
                yield t0, tn, tn // 128
                t0 += tn

        # (reps>1 repeats the whole 3-layer pipeline for slope-based timing;
        # results are idempotent since rep>0 re-reads the same x inputs)
        for _rep, l in ((r, ll) for r in range(reps) for ll in range(NL)):
            hf = hf_all[_rep] if hf_all is not None else None
            gonly = "gatheronly" in (debug or "")
            hcur = x_shard.ap() if l == 0 else shard[l - 1][:]

            def win_src(w):
                if l == 0 or gonly:
                    return x_full.ap()[w * cfg.wlen : w * cfg.wlen + wrows[w], :]
                if cfg.nsplit > 1:
                    if cfg.ag_bf16:
                        # upcast this quarter's AllGather result to f32 rows;
                        # later quarters' collectives still run concurrently
                        nc.gpsimd.dma_start(
                            out=hf[l - 1][w * cfg.wlen : (w + 1) * cfg.wlen, :],
                            in_=hfq_all[_rep][l - 1][w][:],
                        )
                        return hf[l - 1][w * cfg.wlen : w * cfg.wlen + wrows[w], :]
                    return hfq_all[_rep][l - 1][w][: wrows[w], :]
                return hf[l - 1][w * cfg.wlen : w * cfg.wlen + wrows[w], :]

            # ---- gather + segment-sum, per source window ----
            QCOLS = cfg.qcols  # idx cols per SWDGE call (ring-capacity bound)
            for w in range(0 if "mlponly" in (debug or "") else cfg.nwin):
                win = win_src(w)
                asm = asmp.tile([P, cfg.groups * C], f32, name="asm", tag="asm")
                ioff = 0
                for (g0, ds, padc) in sched.chunks[w]:
                    cols = sum(ds) + padc
                    it = idxp.tile([P, cols * 8], i16, name="it", tag="it")
                    nc.sync.dma_start(
                        out=it[:], in_=gidx_t[w].ap()[:, ioff : ioff + cols * 8]
                    )
                    ioff += cols * 8
                    T = gat.tile([P, cols * C], f32, name="gt", tag="gt")
                    co = 0
                    while co < cols:
                        nco = min(QCOLS, cols - co)
                        nc.gpsimd.dma_gather(
                            out_ap=T[:, co * C : (co + nco) * C].rearrange(
                                "p (k f) -> p k f", f=C
                            ),
                            in_ap=win,
                            idxs_ap=it[:, co * 8 : (co + nco) * 8],
                            num_idxs=128 * nco,
                            num_idxs_reg=128 * nco,
                            elem_size=C,
                            queue_num=nextq(),
                        )
                        co += nco
                    # segment-sum each group's slots (strided reduce over d)
                    goff = 0
                    for gi, dd in enumerate(ds):
                        nc.vector.tensor_reduce(
                            out=asm[:, (g0 + gi) * C : (g0 + gi + 1) * C],
                            in_=T[:, goff * C : (goff + dd) * C].rearrange(
                                "p (d f) -> p f d", f=C
                            ),
                            axis=mybir.AxisListType.X,
                            op=add,
                        )
                        goff += dd
                if sched.gmax[w] < cfg.groups:
                    nc.vector.memset(asm[:, sched.gmax[w] * C :], 0.0)
                # store token-ordered partials; the MLP phase gathers them
                # back by node (no serialized scatter-add chain needed)
                nc.sync.dma_start(
                    out=bufw[w][:].rearrange("(k p) f -> p k f", p=P),
                    in_=asm[:].rearrange("p (k f) -> p k f", f=C),
                )

            if gonly:  # timing ablation: gathers+reduces+bufw only, all layers
                continue

            # ---- m = h + Σ_w agg_w (gather-back) ; MLP ----
            def emit_quarter(q):
                # zero pad rows, downcast, and launch this quarter's AllGather
                # as soon as its MLP tiles are stored — later quarters overlap
                qsl = slice(q * cfg.qrows, (q + 1) * cfg.qrows)
                nzq = cfg.qrows - cfg.qreal
                nc.sync.dma_start(
                    out=shard[l][q * cfg.qrows + cfg.qreal : (q + 1) * cfg.qrows, :],
                    in_=zeros[:nzq, :C],
                )
                if cfg.ag_bf16:
                    nc.gpsimd.dma_start(out=sh16[l][qsl, :], in_=shard[l][qsl, :])
                    agin = sh16[l][qsl, :]
                else:
                    agin = shard[l][qsl, :]
                if "fakecc" in (debug or ""):
                    nc.sync.dma_start(
                        out=hfq_all[_rep][l][q][: cfg.qrows, :], in_=agin
                    )
                else:
                    nc.gpsimd.collective_compute(
                        "AllGather",
                        mybir.AluOpType.bypass,
                        replica_groups=rg,
                        ins=[agin],
                        outs=[hfq_all[_rep][l][q][:]],
                    )

            qdone = 0
            BN = 1024  # nodes per agg-gather call (= the SWDGE call cap)
            b0 = 0
            while b0 < cfg.nsh:
                bn = min(BN, cfg.nsh - b0)
                Gs = []
                if "mlponly" not in (debug or ""):
                    for w in range(cfg.nwin):
                        G = mlp.tile(
                            [P, (BN // 128) * C], f32, name=f"G{w}", tag=f"G{w}"
                        )
                        nc.gpsimd.dma_gather(
                            out_ap=G[:, : (bn // 128) * C].rearrange(
                                "p (k f) -> p k f", f=C
                            ),
                            in_ap=bufw[w][:],
                            idxs_ap=sis[w][:, b0 // 16 : (b0 + bn) // 16],
                            num_idxs=bn,
                            num_idxs_reg=bn,
                            elem_size=C,
                            queue_num=nextq(),
                        )
                        Gs.append(G)
                for s0 in range(0, bn, cfg.tile_n):
                    t0 = b0 + s0
                    tn = min(cfg.tile_n, bn - s0)
                    cc = tn // 128
                    ksl = slice((s0 // 128) * C, (s0 // 128 + cc) * C)
                    A = mlp.tile([P, cc_full * C], f32, name="A", tag="A")
                    H = mlp.tile([P, cc_full * C], f32, name="H", tag="H")
                    nc.sync.dma_start(
                        out=H[:, : cc * C].rearrange("p (k f) -> p k f", f=C),
                        in_=hcur[t0 : t0 + tn, :].rearrange("(k p) f -> p k f", p=P),
                    )
                    if Gs:
                        if len(Gs) == 1:
                            nc.vector.tensor_tensor(
                                out=A[:, : cc * C], in0=Gs[0][:, ksl],
                                in1=H[:, : cc * C], op=add,
                            )
                        else:
                            nc.vector.tensor_tensor(
                                out=A[:, : cc * C], in0=Gs[0][:, ksl],
                                in1=Gs[1][:, ksl], op=add,
                            )
                            for G in Gs[2:]:
                                nc.vector.tensor_tensor(
                                    out=A[:, : cc * C], in0=A[:, : cc * C],
                                    in1=G[:, ksl], op=add,
                                )
                            nc.vector.tensor_tensor(
                                out=A[:, : cc * C], in0=A[:, : cc * C],
                                in1=H[:, : cc * C], op=add,
                            )
                    else:
                        nc.vector.tensor_copy(out=A[:, : cc * C], in_=H[:, : cc * C])
                    if "aggonly" in (debug or ""):
                        nc.sync.dma_start(
                            out=out_t.ap()[t0 : t0 + tn, :].rearrange(
                                "(k p) f -> p k f", p=P
                            ),
                            in_=A[:, : cc * C].rearrange("p (k f) -> p k f", f=C),
                        )
                        continue
                    mT = mlp.tile([C, cfg.tile_n], f32, name="mT", tag="mT")
                    transpose_in(A[:], mT[:], cc)
                    Y = psum.tile([C, cfg.tile_n], f32, name="Y", tag="Y")
                    nc.tensor.matmul(
                        out=Y[:, :tn], lhsT=w1s[l][:], rhs=mT[:, :tn],
                        start=True, stop=True,
                    )
                    Ys = mlp.tile([C, cfg.tile_n], f32, name="Ys", tag="Ys")
                    nc.scalar.activation(
                        out=Ys[:, :tn], in_=Y[:, :tn], func=relu, bias=b1s[l][:]
                    )
                    Z = psum.tile([C, cfg.tile_n], f32, name="Z", tag="Y")
                    nc.tensor.matmul(
                        out=Z[:, :tn], lhsT=w2s[l][:], rhs=Ys[:, :tn],
                        start=True, stop=True,
                    )
                    Hn = mlp.tile([C, cfg.tile_n], f32, name="Hn", tag="Hn")
                    nc.scalar.activation(
                        out=Hn[:, :tn], in_=Z[:, :tn], func=relu, bias=b2s[l][:]
                    )
                    if cfg.jk_fuse and l == NL - 1:
                        # fused JumpingKnowledge: Hn IS shard[2] channel-major;
                        # accumulate all three lin_W blocks into one PSUM tile
                        acc = psum.tile([C, cfg.tile_n], f32, name="acc", tag="Y")
                        for ll in range(NL - 1):
                            sT = mlp.tile([C, cfg.tile_n], f32, name="sT", tag="sT")
                            nc.sync.dma_start(
                                out=sT[:, :tn], in_=sh_cm[ll][:, t0 : t0 + tn]
                            )
                            nc.tensor.matmul(
                                out=acc[:, :tn], lhsT=lws[ll][:], rhs=sT[:, :tn],
                                start=(ll == 0), stop=False,
                            )
                        nc.tensor.matmul(
                            out=acc[:, :tn], lhsT=lws[NL - 1][:], rhs=Hn[:, :tn],
                            start=False, stop=True,
                        )
                        O = mlp.tile([C, cfg.tile_n], f32, name="O", tag="Hn")
                        nc.scalar.activation(
                            out=O[:, :tn], in_=acc[:, :tn], func=relu, bias=lb[:]
                        )
                        Om = mlp.tile([P, cc_full * C], f32, name="Om", tag="Hm")
                        transpose_out(O[:], Om[:], cc)
                        nc.sync.dma_start(
                            out=out_t.ap()[t0 : t0 + tn, :].rearrange(
                                "(k p) f -> p k f", p=P
                            ),
                            in_=Om[:, : cc * C].rearrange("p (k f) -> p k f", f=C),
                        )
                        continue
                    if cfg.jk_fuse:
                        nc.sync.dma_start(
                            out=sh_cm[l][:, t0 : t0 + tn], in_=Hn[:, :tn]
                        )
                    Hm = mlp.tile([P, cc_full * C], f32, name="Hm", tag="Hm")
                    transpose_out(Hn[:], Hm[:], cc)
                    nc.sync.dma_start(
                        out=shard[l][t0 : t0 + tn, :].rearrange("(k p) f -> p k f", p=P),
                        in_=Hm[:, : cc * C].rearrange("p (k f) -> p k f", f=C),
                    )
                b0 += bn
                if cfg.nsplit > 1 and l < NL - 1 and "aggonly" not in (debug or ""):
                    while qdone < cfg.nsplit and (qdone + 1) * cfg.qrows <= b0:
                        emit_quarter(qdone)
                        qdone += 1
            if "aggonly" in (debug or ""):
                break
            if cfg.nsplit > 1:
                # quarters already zeroed + gathered in-loop (except last layer,
                # whose pad rows only feed discarded JK outputs)
                continue

            # zero the pad rows, then replicate the shard to every core
            nc.sync.dma_start(
                out=shard[l][cfg.nsh_real : cfg.nsh, :], in_=zeros[:npad, :C]
            )
            if l < NL - 1:
                if "fakecc" in (debug or ""):
                    # timeline-sim mode: stand in for the AllGather with a
                    # local DMA of similar cost (TimelineSim can't do CC)
                    nc.sync.dma_start(
                        out=hf[l][: cfg.nsh, :], in_=shard[l][:]
                    )
                elif cfg.ag_bf16:
                    # halve collective bytes: cast f32->bf16, AllGather bf16,
                    # cast back to 256B f32 rows for the gather source
                    nc.gpsimd.dma_start(out=sh16[l][:], in_=shard[l][:])
                    nc.gpsimd.collective_compute(
                        "AllGather",
                        mybir.AluOpType.bypass,
                        replica_groups=rg,
                        ins=[sh16[l][:]],
                        outs=[hf16_all[_rep][l][:]],
                    )
                    nc.gpsimd.dma_start(out=hf[l][:], in_=hf16_all[_rep][l][:])
                else:
                    nc.gpsimd.collective_compute(
                        "AllGather",
                        mybir.AluOpType.bypass,
                        replica_groups=rg,
                        ins=[shard[l][:]],
                        outs=[hf[l][:]],
                    )

        # ---- JumpingKnowledge cat + final Linear + ReLU ----
        skip_jk = (
            cfg.jk_fuse
            or "aggonly" in (debug or "")
            or "gatheronly" in (debug or "")
        )
        for t0, tn, cc in node_tiles() if not skip_jk else []:
            acc = psum.tile([C, cfg.tile_n], f32, name="acc", tag="Y")
            for l in range(NL):
                S = mlp.tile([P, cc_full * C], f32, name="S", tag="A")
                nc.sync.dma_start(
                    out=S[:, : cc * C].rearrange("p (k f) -> p k f", f=C),
                    in_=shard[l][t0 : t0 + tn, :].rearrange("(p k) f -> p k f", p=P),
                )
                sT = mlp.tile([C, cfg.tile_n], f32, name="sT", tag="mT")
                transpose_in(S[:], sT[:], cc)
                nc.tensor.matmul(
                    out=acc[:, :tn],
                    lhsT=lws[l][:],
                    rhs=sT[:, :tn],
                    start=(l == 0),
                    stop=(l == NL - 1),
                )
            O = mlp.tile([C, cfg.tile_n], f32, name="O", tag="Hn")
            nc.scalar.activation(out=O[:, :tn], in_=acc[:, :tn], func=relu, bias=lb[:])
            Om = mlp.tile([P, cc_full * C], f32, name="Om", tag="Hm")
            transpose_out(O[:], Om[:], cc)
            nc.sync.dma_start(
                out=out_t.ap()[t0 : t0 + tn, :].rearrange("(p k) f -> p k f", p=P),
                in_=Om[:, : cc * C].rearrange("p (k f) -> p k f", f=C),
            )

    nc.compile()
    return nc


# --------------------------------------------------------------------------- #
# host orchestration
# --------------------------------------------------------------------------- #
def make_in_maps(cfg: Cfg, gidx, sidx, x, weights):
    xp = np.zeros((cfg.ntot, cfg.c), np.float32)
    xsh = []
    j = np.arange(cfg.nsh_real)
    lr = _local_row(j, cfg)
    for c in range(cfg.ncores):
        xs = np.zeros((cfg.nsh, cfg.c), np.float32)
        xs[lr] = x[c * cfg.nsh_real : (c + 1) * cfg.nsh_real]
        xsh.append(xs)
        if cfg.nsplit > 1:
            cp = (j // cfg.qreal) * cfg.wlen + c * cfg.qrows + j % cfg.qreal
            xp[cp] = x[c * cfg.nsh_real : (c + 1) * cfg.nsh_real]
        else:
            xp[c * cfg.nsh : c * cfg.nsh + cfg.nsh_real] = x[
                c * cfg.nsh_real : (c + 1) * cfg.nsh_real
            ]
    in_maps = []
    for c in range(cfg.ncores):
        m = {
            "x_full": xp,
            "x_shard": xsh[c],
        }
        for w in range(cfg.nwin):
            m[f"gidx_w{w}"] = gidx[c][w]
            m[f"sidx_w{w}"] = sidx[c][w]
        for l in range(cfg.n_layers):
            m[f"W1_{l}"] = weights[f"W1_{l}"]
            m[f"b1_{l}"] = weights[f"b1_{l}"].reshape(cfg.c, 1)
            m[f"W2_{l}"] = weights[f"W2_{l}"]
            m[f"b2_{l}"] = weights[f"b2_{l}"].reshape(cfg.c, 1)
        m["lin_W"] = weights["lin_W"]
        m["lin_b"] = weights["lin_b"].reshape(cfg.c, 1)
        in_maps.append(m)
    return in_maps


def assemble_output(cfg: Cfg, results):
    out = np.empty((cfg.n, cfg.c), np.float32)
    lr = _local_row(np.arange(cfg.nsh_real), cfg)
    for c in range(cfg.ncores):
        out[c * cfg.nsh_real : (c + 1) * cfg.nsh_real] = results[c]["out_shard"][lr]
    return out


def run_on_hw(nc, in_maps, cfg: Cfg, trace=False):
    from concourse.bass_utils import run_bass_kernel_spmd

    res = run_bass_kernel_spmd(
        nc, in_maps, core_ids=list(range(cfg.ncores)), trace=trace
    )
    return res


def kernel(**inputs) -> np.ndarray:
    x = np.asarray(inputs["x"], np.float32)
    edge_index = np.asarray(inputs["edge_index"])
    cfg = Cfg()
    assert x.shape == (cfg.n, cfg.c)
    sched, gidx, sidx, wrows, pad = preprocess(edge_index, cfg)
    nc = build_program(cfg, sched, wrows)
    in_maps = make_in_maps(cfg, gidx, sidx, x, inputs)
    res = run_on_hw(nc, in_maps, cfg)
    return assemble_output(cfg, res.results)



# revision 36
# speedup vs baseline: 1.7093x; 1.7093x over previous
"""GIN message-passing (3 layers + JumpingKnowledge cat + Linear) on 8 TRN2 NeuronCores.

Strategy (graph/data parallel, edges sharded by destination node):
  - Nodes are partitioned into 8 shards of 12544 rows (4 quarters of 3136 =
    3125 real + 11 pad rows each).  Core c computes the aggregation + MLP for
    its own dst shard and gathers source features from a replicated copy of h.
  - h replication is pipelined per quarter: as soon as a quarter's MLP tiles
    are stored, the quarter is downcast to bf16 and AllGathered (halving the
    collective bytes); the next layer upcasts each quarter back to f32 rows
    right before gathering from it, so later quarters' collectives overlap the
    next layer's gather phase.
  - Source windows == quarters (8*3136 = 25088 rows, int16-addressable).  The
    segment_sum gather uses the GPSIMD dma_gather primitive (int16 indices,
    256B rows); per (core, window) the dst nodes are sorted by in-degree and
    packed into groups of 128 with per-chunk-uniform slot counts, padding
    slots pointing at an all-zero row of h (pad 1.07).
  - Gathered tiles [128 nodes, D slots, 64ch] are segment-summed on the Vector
    engine (strided reduce over the slot axis), stored token-ordered to DRAM,
    and gathered back by node rank in the MLP phase.
  - MLP runs channel-major on the TensorEngine (transpose via PE identity
    matmul), biases+ReLU on the Scalar engine.  Layers 0/1 also store their
    channel-major activations; layer 2 folds the JumpingKnowledge concat +
    final Linear directly into its MLP tiles as a 3-matmul PSUM accumulation
    (two DMA-loaded channel-major operands + the in-flight layer-2 output).
"""

import os
import sys

os.environ.setdefault("MYCRO_LOCAL_CACHE", "1")
if "/opt/trn_rl_repo" not in sys.path:
    sys.path.insert(0, "/opt/trn_rl_repo")

from contextlib import ExitStack
from dataclasses import dataclass, field

import numpy as np


# --------------------------------------------------------------------------- #
# configuration
# --------------------------------------------------------------------------- #
@dataclass
class Cfg:
    n: int = 100000          # real nodes
    e: int = 1600000
    c: int = 64              # channels (in == hid == 64)
    ncores: int = 8
    window: int = 32768      # int16-addressable row window
    cols_max: int = 96       # gather-chunk column budget (slots per partition)
    qcols: int = 8          # idx columns per SWDGE gather call (128*qcols idxs)
    scratch: int = 16384     # SWDGE descriptor carveout bytes/partition
    tile_n: int = 512        # MLP node-tile width
    n_layers: int = 3
    nqueues: int = 4         # SWDGE queues; round-robin unlocks SDMA parallelism
    ag_bf16: bool = True     # AllGather in bf16 with cast DMAs around it
    nsplit: int = 4          # >1: shard quarters define windows; AG per quarter
    jk_fuse: bool = True     # fold the final Linear into layer-2's MLP tiles

    nsh_real: int = field(init=False)
    groups: int = field(init=False)
    nsh: int = field(init=False)
    ntot: int = field(init=False)
    nwin: int = field(init=False)
    qreal: int = field(init=False)
    qrows: int = field(init=False)
    wlen: int = field(init=False)

    def __post_init__(self):
        assert self.n % self.ncores == 0
        self.nsh_real = self.n // self.ncores
        self.groups = -(-self.nsh_real // 128)
        self.nsh = self.groups * 128
        self.ntot = self.nsh * self.ncores
        if self.nsplit > 1:
            S = self.nsplit
            assert self.nsh_real % S == 0 and self.nsh % S == 0
            self.qreal = self.nsh_real // S
            self.qrows = self.nsh // S
            self.wlen = self.ncores * self.qrows
            assert self.wlen <= 32768, "window must stay int16-addressable"
            assert self.qrows > self.qreal, "each quarter needs a zero row"
            self.nwin = S
        else:
            self.qreal = self.qrows = 0
            self.wlen = self.window
            self.nwin = -(-self.ntot // self.window)
        assert self.nsh % 16 == 0
        assert self.nsh > self.nsh_real, "need pad rows to host the zero rows"
        assert self.tile_n % 128 == 0


@dataclass
class Sched:
    """Cross-core-uniform gather schedule.

    chunks[w] = list of (g0, [D_g ...]) — consecutive active groups packed into
    one SBUF tile; gather calls are split along the chunk's columns in pieces
    of <= QCOLS (descriptor-ring cap).  gmax[w] = number of active groups.
    """
    chunks: list
    gmax: list
    totc: list    # per window: total idx columns (sum nidx/16)
    zr: list      # per window: absolute canonical position of an all-zero row


# --------------------------------------------------------------------------- #
# host-side preprocessing
# --------------------------------------------------------------------------- #
def _wrap_idx(v: np.ndarray) -> np.ndarray:
    """int16 vector (len % 16 == 0) -> [128, len/16] wrapped+replicated layout.

    Logical index j lives at [j % 16, j // 16]; the 16-partition pattern is
    replicated to all 128 partitions (ucode cores each read their own stripe).
    """
    w16 = v.reshape(-1, 16).T  # [16, len/16]
    return np.ascontiguousarray(np.tile(w16, (8, 1)))


def _local_row(j: np.ndarray, cfg: Cfg):
    """shard-local node index -> shard row (quarter layout when nsplit>1)."""
    if cfg.nsplit > 1:
        return (j // cfg.qreal) * cfg.qrows + j % cfg.qreal
    return j


def preprocess(edge_index: np.ndarray, cfg: Cfg):
    nc_, nw, nsh = cfg.ncores, cfg.nwin, cfg.nsh
    wlen = cfg.wlen
    src = edge_index[0].astype(np.int64)
    dst = edge_index[1].astype(np.int64)

    c_of = dst // cfg.nsh_real
    ld = _local_row(dst % cfg.nsh_real, cfg)                  # local dst row
    if cfg.nsplit > 1:
        # canonical pos: [quarter][core][row-in-quarter]
        cs = src // cfg.nsh_real
        j = src % cfg.nsh_real
        ps = (j // cfg.qreal) * wlen + cs * cfg.qrows + j % cfg.qreal
    else:
        ps = (src // cfg.nsh_real) * nsh + src % cfg.nsh_real
    w_of = ps // wlen

    key = (c_of * nw + w_of) * nsh + ld
    order = np.lexsort((ps, key))
    key_s = key[order]
    ps_s = ps[order]
    cnt = np.bincount(key_s, minlength=nc_ * nw * nsh).reshape(nc_, nw, nsh)
    base = np.zeros(nc_ * nw * nsh + 1, np.int64)
    base[1:] = np.cumsum(cnt.ravel())

    # window row counts + zero rows (pad rows stay zero every layer)
    if cfg.nsplit > 1:
        wrows = [wlen] * nw
        pad_pos = np.concatenate(
            [
                q * wlen + c * cfg.qrows + np.arange(cfg.qreal, cfg.qrows)
                for q in range(cfg.nsplit)
                for c in range(nc_)
            ]
        )
    else:
        wrows = [min(wlen, cfg.ntot - w * wlen) for w in range(nw)]
        pad_pos = np.concatenate(
            [np.arange(c * nsh + cfg.nsh_real, (c + 1) * nsh) for c in range(nc_)]
        )
    zr = []
    for w in range(nw):
        cand = pad_pos[(pad_pos >= w * wlen) & (pad_pos < w * wlen + wrows[w])]
        assert cand.size, f"window {w} has no zero row"
        zr.append(int(cand[0]))

    # per-(core,window) degree-sorted node order
    pi = np.empty((nc_, nw, nsh), np.int64)
    for c in range(nc_):
        for w in range(nw):
            pi[c, w] = np.argsort(-cnt[c, w], kind="stable")

    # shared schedule: per-group D = cross-core max of group max degree
    D = np.zeros((nw, cfg.groups), np.int64)
    for w in range(nw):
        sd = -np.sort(-cnt[:, w, :], axis=1)       # per-core sorted desc
        D[w] = sd[:, ::128].max(axis=0)
    chunks, gmax, totc = [], [], []
    for w in range(nw):
        gm = int(np.count_nonzero(D[w]))           # actives are a prefix
        ch, g0 = [], 0
        while g0 < gm:
            ds, g1 = [], g0
            while g1 < gm and sum(ds) + D[w][g1] <= cfg.cols_max:
                ds.append(int(D[w][g1]))
                g1 += 1
            if not ds:  # single group larger than cols_max
                ds = [int(D[w][g0])]
                g1 = g0 + 1
            # pad to a multiple of 8 columns so every SWDGE call is a full
            # 1024 descriptors (padding columns gather the zero row)
            padc = (-sum(ds)) % 8
            ch.append((g0, ds, padc))
            g0 = g1
        chunks.append(ch)
        gmax.append(gm)
        totc.append(sum(128 * (sum(ds) + padc) // 16 for (_, ds, padc) in ch))

    # per-core gather / scatter index arrays
    gidx = [[None] * nw for _ in range(nc_)]
    sidx = [[None] * nw for _ in range(nc_)]
    tot_slots = 0
    for c in range(nc_):
        for w in range(nw):
            parts = []
            zr_loc = zr[w] - w * wlen
            for (g0, ds, padc) in chunks[w]:
                cols = sum(ds)
                nidx = 128 * cols
                col2g = np.repeat(np.arange(len(ds)), ds)         # chunk-local g
                col2d = np.concatenate([np.arange(dd) for dd in ds])
                i = np.arange(nidx)
                p = i & 127
                col = i >> 7
                r = (g0 + col2g[col]) * 128 + p
                d = col2d[col]
                node = pi[c, w][r]
                deg = cnt[c, w, node]
                k = (c * nw + w) * nsh + node
                e = base[k] + np.minimum(d, np.maximum(deg - 1, 0))
                e = np.minimum(e, max(len(ps_s) - 1, 0))  # empty tail segments
                v = ps_s[e] - w * wlen
                v = np.where(d < deg, v, zr_loc)
                if padc:
                    v = np.concatenate([v, np.full(128 * padc, zr_loc, np.int64)])
                assert v.min() >= 0 and v.max() < wrows[w]
                parts.append(_wrap_idx(v.astype(np.int16)))
                tot_slots += nidx + 128 * padc
            gidx[c][w] = np.concatenate(parts, axis=1)
            rank = np.empty(nsh, np.int64)
            rank[pi[c, w]] = np.arange(nsh)  # node -> token position
            sidx[c][w] = _wrap_idx(rank.astype(np.int16))

    pad_frac = tot_slots / max(len(src), 1)
    return Sched(chunks, gmax, totc, zr), gidx, sidx, wrows, pad_frac


# --------------------------------------------------------------------------- #
# device program
# --------------------------------------------------------------------------- #
def _patch_queue_affine_sem_lanes():
    """Make Tile's DMASW lane assignment queue-affine.

    SWDGE completion sems are locked to the queue that first increments them
    (ucode sem_target is per-queue).  Tile cycles lanes 0..7 in scheduled
    order, which breaks once calls use queue_num 1..3.  Map queue q to lanes
    {q, q+4} so every lane only ever serves one queue.
    """
    import concourse.tile_sem_assignment as tsa
    import concourse.bass_isa as bass_isa
    import concourse.mybir as mybir

    if getattr(tsa, "_queue_affine_patched", False):
        return
    orig = tsa.TileClockTick._assign_tick
    DMAInst = tsa.DMAInst

    def _assign_tick(self, inst):
        if (
            inst.engine == mybir.EngineType.Pool
            and isinstance(inst, DMAInst)
            and not isinstance(inst, bass_isa.UserSyncedRemoteDMADescs)
            and self.swdge_sem_count == 8
        ):
            q = int(getattr(inst, "queue_num", 0) or 0) % 4
            tog = getattr(self, "_q_lane_toggle", None)
            if tog is None:
                tog = self._q_lane_toggle = {}
            t = tog.get(q, 0)
            tog[q] = t ^ 1
            self.next_sw_dma_idx = q + 4 * t
        return orig(self, inst)

    tsa.TileClockTick._assign_tick = _assign_tick
    tsa._queue_affine_patched = True


def build_program(cfg: Cfg, sched: Sched, wrows, debug=None, reps=1):
    import concourse.bacc as bacc
    import concourse.mybir as mybir
    import concourse.tile as tile
    from concourse.masks import make_identity

    if cfg.nqueues > 1:
        _patch_queue_affine_sem_lanes()

    f32 = mybir.dt.float32
    i16 = mybir.dt.int16
    C = cfg.c
    P = 128
    NL = cfg.n_layers

    nc = bacc.Bacc(
        "TRN2",
        target_bir_lowering=False,
        debug=False,
        num_devices=cfg.ncores,
        num_swdge_queues=cfg.nqueues,
        dynamic_dma_scratch_size=cfg.scratch,
    )
    qrr = iter(range(10**9))  # round-robin SWDGE queue counter

    def nextq():
        return next(qrr) % cfg.nqueues

    x_full = nc.dram_tensor("x_full", [cfg.ntot, C], f32, kind="ExternalInput")
    x_shard = nc.dram_tensor("x_shard", [cfg.nsh, C], f32, kind="ExternalInput")
    gidx_t = [
        nc.dram_tensor(f"gidx_w{w}", [P, sched.totc[w]], i16, kind="ExternalInput")
        for w in range(cfg.nwin)
    ]
    sidx_t = [
        nc.dram_tensor(f"sidx_w{w}", [P, cfg.nsh // 16], i16, kind="ExternalInput")
        for w in range(cfg.nwin)
    ]
    w1_t, b1_t, w2_t, b2_t = [], [], [], []
    for l in range(NL):
        w1_t.append(nc.dram_tensor(f"W1_{l}", [C, C], f32, kind="ExternalInput"))
        b1_t.append(nc.dram_tensor(f"b1_{l}", [C, 1], f32, kind="ExternalInput"))
        w2_t.append(nc.dram_tensor(f"W2_{l}", [C, C], f32, kind="ExternalInput"))
        b2_t.append(nc.dram_tensor(f"b2_{l}", [C, 1], f32, kind="ExternalInput"))
    linw_t = nc.dram_tensor("lin_W", [NL * C, C], f32, kind="ExternalInput")
    linb_t = nc.dram_tensor("lin_b", [C, 1], f32, kind="ExternalInput")
    out_t = nc.dram_tensor("out_shard", [cfg.nsh, C], f32, kind="ExternalOutput")

    rg = [list(range(cfg.ncores))]
    add = mybir.AluOpType.add
    relu = mybir.ActivationFunctionType.Relu

    with tile.TileContext(nc) as tc, ExitStack() as ctx:
        const = ctx.enter_context(tc.tile_pool(name="const", bufs=1))
        idxp = ctx.enter_context(tc.tile_pool(name="idx", bufs=3))
        gat = ctx.enter_context(tc.tile_pool(name="gat", bufs=3))
        asmp = ctx.enter_context(tc.tile_pool(name="asm", bufs=2))
        mlp = ctx.enter_context(tc.tile_pool(name="mlp", bufs=2))
        psum = ctx.enter_context(tc.tile_pool(name="psum", bufs=2, space="PSUM"))
        dram = ctx.enter_context(tc.tile_pool(name="dram", bufs=1, space="DRAM"))

        # ---- constants ----
        identity = const.tile([P, P], f32)
        make_identity(nc, identity[:])
        ZCH = 16  # zero-fill chunk (groups per DMA)
        zeros = const.tile([P, ZCH * C], f32)
        nc.vector.memset(zeros[:], 0.0)
        w1s, b1s, w2s, b2s, lws = [], [], [], [], []
        for l in range(NL):
            w1 = const.tile([C, C], f32, name=f"w1s_{l}")
            nc.sync.dma_start(out=w1[:], in_=w1_t[l].ap())
            w1s.append(w1)
            b1 = const.tile([C, 1], f32, name=f"b1s_{l}")
            nc.sync.dma_start(out=b1[:], in_=b1_t[l].ap())
            b1s.append(b1)
            w2 = const.tile([C, C], f32, name=f"w2s_{l}")
            nc.sync.dma_start(out=w2[:], in_=w2_t[l].ap())
            w2s.append(w2)
            b2 = const.tile([C, 1], f32, name=f"b2s_{l}")
            nc.sync.dma_start(out=b2[:], in_=b2_t[l].ap())
            b2s.append(b2)
            lw = const.tile([C, C], f32, name=f"lws_{l}")
            nc.sync.dma_start(out=lw[:], in_=linw_t.ap()[l * C : (l + 1) * C, :])
            lws.append(lw)
        lb = const.tile([C, 1], f32)
        nc.sync.dma_start(out=lb[:], in_=linb_t.ap())
        sis = []  # node -> token rank per window (agg gather-back indices)
        for w in range(cfg.nwin):
            si = const.tile([P, cfg.nsh // 16], i16, name=f"sis_{w}")
            nc.sync.dma_start(out=si[:], in_=sidx_t[w].ap())
            sis.append(si)

        # ---- internal DRAM ----
        # per-window partial aggregations, in token (degree-sorted) order
        bufw = [dram.tile([cfg.nsh, C], f32, name=f"bufw_{w}") for w in range(cfg.nwin)]
        shard = [dram.tile([cfg.nsh, C], f32, name=f"shard_{l}") for l in range(NL)]
        # channel-major copies of layers 0/1 for the fused JK matmuls
        sh_cm = [
            dram.tile([C, cfg.nsh], f32, name=f"shcm_{l}") for l in range(NL)
        ] if cfg.jk_fuse else None
        # Shared tensors allow a single writer only -> one set per rep
        bf16 = mybir.dt.bfloat16
        hf_all = hf16_all = hfq_all = sh16 = None
        if cfg.nsplit > 1:
            agdt = bf16 if cfg.ag_bf16 else f32
            hfq_all = [
                [
                    [
                        dram.tile(
                            [cfg.wlen, C], agdt, name=f"hfq_{r}_{l}_{q}",
                            addr_space="Shared",
                        )
                        for q in range(cfg.nsplit)
                    ]
                    for l in range(NL - 1)
                ]
                for r in range(reps)
            ]
            if cfg.ag_bf16:
                hf_all = [
                    [
                        dram.tile([cfg.ntot, C], f32, name=f"hf_{r}_{l}")
                        for l in range(NL - 1)
                    ]
                    for r in range(reps)
                ]
                sh16 = [
                    dram.tile([cfg.nsh, C], bf16, name=f"sh16_{l}")
                    for l in range(NL - 1)
                ]
        elif cfg.ag_bf16:
            hf_all = [
                [
                    dram.tile([cfg.ntot, C], f32, name=f"hf_{r}_{l}")
                    for l in range(NL - 1)
                ]
                for r in range(reps)
            ]
            hf16_all = [
                [
                    dram.tile(
                        [cfg.ntot, C], bf16, name=f"hf16_{r}_{l}", addr_space="Shared"
                    )
                    for l in range(NL - 1)
                ]
                for r in range(reps)
            ]
            sh16 = [
                dram.tile([cfg.nsh, C], bf16, name=f"sh16_{l}") for l in range(NL - 1)
            ]
        else:
            hf_all = [
                [
                    dram.tile([cfg.ntot, C], f32, name=f"hf_{r}_{l}", addr_space="Shared")
                    for l in range(NL - 1)
                ]
                for r in range(reps)
            ]

        npad = cfg.nsh - cfg.nsh_real
        cc_full = cfg.tile_n // 128

        def transpose_in(src_ap, dst_ap, cc):
            """node-major [128, cc*C] -> channel-major [C, cc*128]."""
            for s in range(cc):
                pt = psum.tile([C, P], f32, name="tp", tag="tp")
                nc.tensor.transpose(
                    out=pt[:], in_=src_ap[:, s * C : (s + 1) * C], identity=identity[:]
                )
                nc.scalar.copy(out=dst_ap[:, s * P : (s + 1) * P], in_=pt[:])

        def transpose_out(src_ap, dst_ap, cc):
            """channel-major [C, cc*128] -> node-major [128, cc*C]."""
            for s in range(cc):
                pt = psum.tile([P, C], f32, name="tpo", tag="tp")
                nc.tensor.transpose(
                    out=pt[:],
                    in_=src_ap[:, s * P : (s + 1) * P],
                    identity=identity[:C, :C],
                )
                nc.scalar.copy(out=dst_ap[:, s * C : (s + 1) * C], in_=pt[:])

        def node_tiles():
            t0 = 0
            while t0 < cfg.nsh:
                tn = min(cfg.tile_n, cfg.nsh - t0)
                yield t0, tn, tn // 128
                t0 += tn

        # (reps>1 repeats the whole 3-layer pipeline for slope-based timing;
        # results are idempotent since rep>0 re-reads the same x inputs)
        for _rep, l in ((r, ll) for r in range(reps) for ll in range(NL)):
            hf = hf_all[_rep] if hf_all is not None else None
            gonly = "gatheronly" in (debug or "")
            hcur = x_shard.ap() if l == 0 else shard[l - 1][:]

            def win_src(w):
                if l == 0 or gonly:
                    return x_full.ap()[w * cfg.wlen : w * cfg.wlen + wrows[w], :]
                if cfg.nsplit > 1:
                    if cfg.ag_bf16:
                        # upcast this quarter's AllGather result to f32 rows;
                        # later quarters' collectives still run concurrently
                        nc.gpsimd.dma_start(
                            out=hf[l - 1][w * cfg.wlen : (w + 1) * cfg.wlen, :],
                            in_=hfq_all[_rep][l - 1][w][:],
                        )
                        return hf[l - 1][w * cfg.wlen : w * cfg.wlen + wrows[w], :]
                    return hfq_all[_rep][l - 1][w][: wrows[w], :]
                return hf[l - 1][w * cfg.wlen : w * cfg.wlen + wrows[w], :]

            # ---- gather + segment-sum, per source window ----
            QCOLS = cfg.qcols  # idx cols per SWDGE call (ring-capacity bound)
            for w in range(0 if "mlponly" in (debug or "") else cfg.nwin):
                win = win_src(w)
                asm = asmp.tile([P, cfg.groups * C], f32, name="asm", tag="asm")
                ioff = 0
                for (g0, ds, padc) in sched.chunks[w]:
                    cols = sum(ds) + padc
                    it = idxp.tile([P, cols * 8], i16, name="it", tag="it")
                    nc.sync.dma_start(
                        out=it[:], in_=gidx_t[w].ap()[:, ioff : ioff + cols * 8]
                    )
                    ioff += cols * 8
                    T = gat.tile([P, cols * C], f32, name="gt", tag="gt")
                    co = 0
                    while co < cols:
                        nco = min(QCOLS, cols - co)
                        nc.gpsimd.dma_gather(
                            out_ap=T[:, co * C : (co + nco) * C].rearrange(
                                "p (k f) -> p k f", f=C
                            ),
                            in_ap=win,
                            idxs_ap=it[:, co * 8 : (co + nco) * 8],
                            num_idxs=128 * nco,
                            num_idxs_reg=128 * nco,
                            elem_size=C,
                            queue_num=nextq(),
                        )
                        co += nco
                    # segment-sum each group's slots (strided reduce over d)
                    goff = 0
                    for gi, dd in enumerate(ds):
                        nc.vector.tensor_reduce(
                            out=asm[:, (g0 + gi) * C : (g0 + gi + 1) * C],
                            in_=T[:, goff * C : (goff + dd) * C].rearrange(
                                "p (d f) -> p f d", f=C
                            ),
                            axis=mybir.AxisListType.X,
                            op=add,
                        )
                        goff += dd
                if sched.gmax[w] < cfg.groups:
                    nc.vector.memset(asm[:, sched.gmax[w] * C :], 0.0)
                # store token-ordered partials; the MLP phase gathers them
                # back by node (no serialized scatter-add chain needed)
                nc.sync.dma_start(
                    out=bufw[w][:].rearrange("(k p) f -> p k f", p=P),
                    in_=asm[:].rearrange("p (k f) -> p k f", f=C),
                )

            if gonly:  # timing ablation: gathers+reduces+bufw only, all layers
                continue

            # ---- m = h + Σ_w agg_w (gather-back) ; MLP ----
            def emit_quarter(q):
                # zero pad rows, downcast, and launch this quarter's AllGather
                # as soon as its MLP tiles are stored — later quarters overlap
                qsl = slice(q * cfg.qrows, (q + 1) * cfg.qrows)
                nzq = cfg.qrows - cfg.qreal
                nc.sync.dma_start(
                    out=shard[l][q * cfg.qrows + cfg.qreal : (q + 1) * cfg.qrows, :],
                    in_=zeros[:nzq, :C],
                )
                if cfg.ag_bf16:
                    nc.gpsimd.dma_start(out=sh16[l][qsl, :], in_=shard[l][qsl, :])
                    agin = sh16[l][qsl, :]
                else:
                    agin = shard[l][qsl, :]
                if "fakecc" in (debug or ""):
                    nc.sync.dma_start(
                        out=hfq_all[_rep][l][q][: cfg.qrows, :], in_=agin
                    )
                else:
                    nc.gpsimd.collective_compute(
                        "AllGather",
                        mybir.AluOpType.bypass,
                        replica_groups=rg,
                        ins=[agin],
                        outs=[hfq_all[_rep][l][q][:]],
                    )

            qdone = 0
            BN = 1024  # nodes per agg-gather call (= the SWDGE call cap)
            b0 = 0
            while b0 < cfg.nsh:
                bn = min(BN, cfg.nsh - b0)
                Gs = []
                if "mlponly" not in (debug or ""):
                    for w in range(cfg.nwin):
                        G = mlp.tile(
                            [P, (BN // 128) * C], f32, name=f"G{w}", tag=f"G{w}"
                        )
                        nc.gpsimd.dma_gather(
                            out_ap=G[:, : (bn // 128) * C].rearrange(
                                "p (k f) -> p k f", f=C
                            ),
                            in_ap=bufw[w][:],
                            idxs_ap=sis[w][:, b0 // 16 : (b0 + bn) // 16],
                            num_idxs=bn,
                            num_idxs_reg=bn,
                            elem_size=C,
                            queue_num=nextq(),
                        )
                        Gs.append(G)
                for s0 in range(0, bn, cfg.tile_n):
                    t0 = b0 + s0
                    tn = min(cfg.tile_n, bn - s0)
                    cc = tn // 128
                    ksl = slice((s0 // 128) * C, (s0 // 128 + cc) * C)
                    A = mlp.tile([P, cc_full * C], f32, name="A", tag="A")
                    H = mlp.tile([P, cc_full * C], f32, name="H", tag="H")
                    nc.sync.dma_start(
                        out=H[:, : cc * C].rearrange("p (k f) -> p k f", f=C),
                        in_=hcur[t0 : t0 + tn, :].rearrange("(k p) f -> p k f", p=P),
                    )
                    if Gs:
                        if len(Gs) == 1:
                            nc.vector.tensor_tensor(
                                out=A[:, : cc * C], in0=Gs[0][:, ksl],
                                in1=H[:, : cc * C], op=add,
                            )
                        else:
                            nc.vector.tensor_tensor(
                                out=A[:, : cc * C], in0=Gs[0][:, ksl],
                                in1=Gs[1][:, ksl], op=add,
                            )
                            for G in Gs[2:]:
                                nc.vector.tensor_tensor(
                                    out=A[:, : cc * C], in0=A[:, : cc * C],
                                    in1=G[:, ksl], op=add,
                                )
                            nc.vector.tensor_tensor(
                                out=A[:, : cc * C], in0=A[:, : cc * C],
                                in1=H[:, : cc * C], op=add,
                            )
                    else:
                        nc.vector.tensor_copy(out=A[:, : cc * C], in_=H[:, : cc * C])
                    if "aggonly" in (debug or ""):
                        nc.sync.dma_start(
                            out=out_t.ap()[t0 : t0 + tn, :].rearrange(
                                "(k p) f -> p k f", p=P
                            ),
                            in_=A[:, : cc * C].rearrange("p (k f) -> p k f", f=C),
                        )
                        continue
                    mT = mlp.tile([C, cfg.tile_n], f32, name="mT", tag="mT")
                    transpose_in(A[:], mT[:], cc)
                    Y = psum.tile([C, cfg.tile_n], f32, name="Y", tag="Y")
                    nc.tensor.matmul(
                        out=Y[:, :tn], lhsT=w1s[l][:], rhs=mT[:, :tn],
                        start=True, stop=True,
                    )
                    Ys = mlp.tile([C, cfg.tile_n], f32, name="Ys", tag="Ys")
                    nc.scalar.activation(
                        out=Ys[:, :tn], in_=Y[:, :tn], func=relu, bias=b1s[l][:]
                    )
                    Z = psum.tile([C, cfg.tile_n], f32, name="Z", tag="Y")
                    nc.tensor.matmul(
                        out=Z[:, :tn], lhsT=w2s[l][:], rhs=Ys[:, :tn],
                        start=True, stop=True,
                    )
                    Hn = mlp.tile([C, cfg.tile_n], f32, name="Hn", tag="Hn")
                    nc.scalar.activation(
                        out=Hn[:, :tn], in_=Z[:, :tn], func=relu, bias=b2s[l][:]
                    )
                    if cfg.jk_fuse and l == NL - 1:
                        # fused JumpingKnowledge: Hn IS shard[2] channel-major;
                        # accumulate all three lin_W blocks into one PSUM tile
                        acc = psum.tile([C, cfg.tile_n], f32, name="acc", tag="Y")
                        for ll in range(NL - 1):
                            sT = mlp.tile([C, cfg.tile_n], f32, name="sT", tag="sT")
                            nc.sync.dma_start(
                                out=sT[:, :tn], in_=sh_cm[ll][:, t0 : t0 + tn]
                            )
                            nc.tensor.matmul(
                                out=acc[:, :tn], lhsT=lws[ll][:], rhs=sT[:, :tn],
                                start=(ll == 0), stop=False,
                            )
                        nc.tensor.matmul(
                            out=acc[:, :tn], lhsT=lws[NL - 1][:], rhs=Hn[:, :tn],
                            start=False, stop=True,
                        )
                        O = mlp.tile([C, cfg.tile_n], f32, name="O", tag="Hn")
                        nc.scalar.activation(
                            out=O[:, :tn], in_=acc[:, :tn], func=relu, bias=lb[:]
                        )
                        Om = mlp.tile([P, cc_full * C], f32, name="Om", tag="Hm")
                        transpose_out(O[:], Om[:], cc)
                        nc.sync.dma_start(
                            out=out_t.ap()[t0 : t0 + tn, :].rearrange(
                                "(k p) f -> p k f", p=P
                            ),
                            in_=Om[:, : cc * C].rearrange("p (k f) -> p k f", f=C),
                        )
                        continue
                    if cfg.jk_fuse:
                        nc.sync.dma_start(
                            out=sh_cm[l][:, t0 : t0 + tn], in_=Hn[:, :tn]
                        )
                    Hm = mlp.tile([P, cc_full * C], f32, name="Hm", tag="Hm")
                    transpose_out(Hn[:], Hm[:], cc)
                    nc.sync.dma_start(
                        out=shard[l][t0 : t0 + tn, :].rearrange("(k p) f -> p k f", p=P),
                        in_=Hm[:, : cc * C].rearrange("p (k f) -> p k f", f=C),
                    )
                b0 += bn
                if cfg.nsplit > 1 and l < NL - 1 and "aggonly" not in (debug or ""):
                    while qdone < cfg.nsplit and (qdone + 1) * cfg.qrows <= b0:
                        emit_quarter(qdone)
                        qdone += 1
            if "aggonly" in (debug or ""):
                break
            if cfg.nsplit > 1:
                # quarters already zeroed + gathered in-loop (except last layer,
                # whose pad rows only feed discarded JK outputs)
                continue

            # zero the pad rows, then replicate the shard to every core
            nc.sync.dma_start(
                out=shard[l][cfg.nsh_real : cfg.nsh, :], in_=zeros[:npad, :C]
            )
            if l < NL - 1:
                if "fakecc" in (debug or ""):
                    # timeline-sim mode: stand in for the AllGather with a
                    # local DMA of similar cost (TimelineSim can't do CC)
                    nc.sync.dma_start(
                        out=hf[l][: cfg.nsh, :], in_=shard[l][:]
                    )
                elif cfg.ag_bf16:
                    # halve collective bytes: cast f32->bf16, AllGather bf16,
                    # cast back to 256B f32 rows for the gather source
                    nc.gpsimd.dma_start(out=sh16[l][:], in_=shard[l][:])
                    nc.gpsimd.collective_compute(
                        "AllGather",
                        mybir.AluOpType.bypass,
                        replica_groups=rg,
                        ins=[sh16[l][:]],
                        outs=[hf16_all[_rep][l][:]],
                    )
                    nc.gpsimd.dma_start(out=hf[l][:], in_=hf16_all[_rep][l][:])
                else:
                    nc.gpsimd.collective_compute(
                        "AllGather",
                        mybir.AluOpType.bypass,
                        replica_groups=rg,
                        ins=[shard[l][:]],
                        outs=[hf[l][:]],
                    )

        # ---- JumpingKnowledge cat + final Linear + ReLU ----
        skip_jk = (
            cfg.jk_fuse
            or "aggonly" in (debug or "")
            or "gatheronly" in (debug or "")
        )
        for t0, tn, cc in node_tiles() if not skip_jk else []:
            acc = psum.tile([C, cfg.tile_n], f32, name="acc", tag="Y")
            for l in range(NL):
                S = mlp.tile([P, cc_full * C], f32, name="S", tag="A")
                nc.sync.dma_start(
                    out=S[:, : cc * C].rearrange("p (k f) -> p k f", f=C),
                    in_=shard[l][t0 : t0 + tn, :].rearrange("(p k) f -> p k f", p=P),
                )
                sT = mlp.tile([C, cfg.tile_n], f32, name="sT", tag="mT")
                transpose_in(S[:], sT[:], cc)
                nc.tensor.matmul(
                    out=acc[:, :tn],
                    lhsT=lws[l][:],
                    rhs=sT[:, :tn],
                    start=(l == 0),
                    stop=(l == NL - 1),
                )
            O = mlp.tile([C, cfg.tile_n], f32, name="O", tag="Hn")
            nc.scalar.activation(out=O[:, :tn], in_=acc[:, :tn], func=relu, bias=lb[:])
            Om = mlp.tile([P, cc_full * C], f32, name="Om", tag="Hm")
            transpose_out(O[:], Om[:], cc)
            nc.sync.dma_start(
                out=out_t.ap()[t0 : t0 + tn, :].rearrange("(p k) f -> p k f", p=P),
                in_=Om[:, : cc * C].rearrange("p (k f) -> p k f", f=C),
            )

    nc.compile()
    return nc


# --------------------------------------------------------------------------- #
# host orchestration
# --------------------------------------------------------------------------- #
def make_in_maps(cfg: Cfg, gidx, sidx, x, weights):
    xp = np.zeros((cfg.ntot, cfg.c), np.float32)
    xsh = []
    j = np.arange(cfg.nsh_real)
    lr = _local_row(j, cfg)
    for c in range(cfg.ncores):
        xs = np.zeros((cfg.nsh, cfg.c), np.float32)
        xs[lr] = x[c * cfg.nsh_real : (c + 1) * cfg.nsh_real]
        xsh.append(xs)
        if cfg.nsplit > 1:
            cp = (j // cfg.qreal) * cfg.wlen + c * cfg.qrows + j % cfg.qreal
            xp[cp] = x[c * cfg.nsh_real : (c + 1) * cfg.nsh_real]
        else:
            xp[c * cfg.nsh : c * cfg.nsh + cfg.nsh_real] = x[
                c * cfg.nsh_real : (c + 1) * cfg.nsh_real
            ]
    in_maps = []
    for c in range(cfg.ncores):
        m = {
            "x_full": xp,
            "x_shard": xsh[c],
        }
        for w in range(cfg.nwin):
            m[f"gidx_w{w}"] = gidx[c][w]
            m[f"sidx_w{w}"] = sidx[c][w]
        for l in range(cfg.n_layers):
            m[f"W1_{l}"] = weights[f"W1_{l}"]
            m[f"b1_{l}"] = weights[f"b1_{l}"].reshape(cfg.c, 1)
            m[f"W2_{l}"] = weights[f"W2_{l}"]
            m[f"b2_{l}"] = weights[f"b2_{l}"].reshape(cfg.c, 1)
        m["lin_W"] = weights["lin_W"]
        m["lin_b"] = weights["lin_b"].reshape(cfg.c, 1)
        in_maps.append(m)
    return in_maps


def assemble_output(cfg: Cfg, results):
    out = np.empty((cfg.n, cfg.c), np.float32)
    lr = _local_row(np.arange(cfg.nsh_real), cfg)
    for c in range(cfg.ncores):
        out[c * cfg.nsh_real : (c + 1) * cfg.nsh_real] = results[c]["out_shard"][lr]
    return out


def run_on_hw(nc, in_maps, cfg: Cfg, trace=False):
    from concourse.bass_utils import run_bass_kernel_spmd

    res = run_bass_kernel_spmd(
        nc, in_maps, core_ids=list(range(cfg.ncores)), trace=trace
    )
    return res


def kernel(**inputs) -> np.ndarray:
    x = np.asarray(inputs["x"], np.float32)
    edge_index = np.asarray(inputs["edge_index"])
    cfg = Cfg()
    assert x.shape == (cfg.n, cfg.c)
    sched, gidx, sidx, wrows, pad = preprocess(edge_index, cfg)
    nc = build_program(cfg, sched, wrows)
    in_maps = make_in_maps(cfg, gidx, sidx, x, inputs)
    res = run_on_hw(nc, in_maps, cfg)
    return assemble_output(cfg, res.results)

